# revision 9
# baseline (speedup 1.0000x reference)
"""Trainium2 Bass kernel for BasicConvClassifierWithSubject.

Strategy: pure data parallel over 8 cores (128 items/core). Per item the whole
network runs as a chain of f32r matmuls on the PE with BN folded into weights,
gelu on ScalarE, residuals accumulated in PSUM via identity/skip matmuls.
Spatial attention + per-subject 1x1 conv fuse on the host into one per-item
[271,128] stationary matrix (gathered by subject), so the device sees a single
K=271 matmul per item for the whole front end.
"""

import os
import numpy as np

import concourse.bass as bass
import concourse.tile as tile
from concourse import bacc, mybir
from concourse.bass_utils import run_bass_kernel_spmd

f32 = mybir.dt.float32
f32r = mybir.dt.float32r
AF = mybir.ActivationFunctionType
AX = mybir.AxisListType

# problem constants (hardcoded per harness contract)
B, C, T = 1024, 271, 281
H, H2, E, NCLS, NSUBJ = 128, 256, 16, 1854, 4
EPS = 1e-5
TP = 284          # padded time length per chunk (zeros at cols 0, 282, 283)
N = 282           # matmul moving size (even, covers cols t..t+281 of pad)
KC = 3            # K chunks for the fused front matmul (271 -> 128+128+15)

_CACHE = {}


def _build(n_items, n_cores):
    """Build + compile the bass program for n_items items per core."""
    nc = bacc.Bacc("TRN2", target_bir_lowering=False, debug=False,
                   num_devices=n_cores)

    def din(name, shape, dt=f32r):
        return nc.dram_tensor(name, shape, dt, kind="ExternalInput").ap()

    X = din("X", [n_items, C, T])
    Mg = din("Mg", [n_items, 128, KC * 128])
    D = din("D", [128, n_items], f32)           # per-item h0 bias columns
    cbias = din("cbias", [128, 10], f32)        # stage bias columns
    w_b1c1 = din("w_b1c1", [128, 3 * 128])
    w_b1c2 = din("w_b1c2", [128, 3 * 128])
    w_b2c1 = din("w_b2c1", [128, 6 * 128])      # (k,h) -> col (k*2+h)
    w_b2sk = din("w_b2sk", [128, 2 * 128])
    w_b2c2 = din("w_b2c2", [128, 12 * 128])     # (k,c,h) -> ((k*2+c)*2+h)
    w_b3c1 = din("w_b3c1", [128, 12 * 128])
    w_b3c2 = din("w_b3c2", [128, 12 * 128])
    ident = din("ident", [128, 128])
    w1 = din("w1", [128, 2 * 128])              # head1 lhsT chunks (V chunks)
    w1x = din("w1x", [128, 128])                # emb rows + head_b1 row
    rhsx = din("rhsx", [128, n_items], )        # emb gather cols + ones row
    w2t = din("w2t", [128, NCLS])
    b2row = din("b2row", [1, NCLS])
    ones1 = din("ones1", [1, n_items])
    out = nc.dram_tensor("out", [n_items, NCLS], f32, kind="ExternalOutput").ap()

    PAR = 2  # pipeline parity (double-buffered persistent activation tiles)

    with tile.TileContext(nc) as tc:
        wpool = tc.alloc_tile_pool(name="w", bufs=1)
        apool = tc.alloc_tile_pool(name="a", bufs=1)
        iopool = tc.alloc_tile_pool(name="io", bufs=3)
        pspool = tc.alloc_tile_pool(name="ps", bufs=1, space="PSUM")

        # resident weights
        def wtile(ap_, name):
            t = wpool.tile(list(ap_.shape), ap_.dtype, tag=name, name=name)
            nc.sync.dma_start(t[:], ap_[:])
            return t

        tw_b1c1 = wtile(w_b1c1, "w_b1c1")
        tw_b1c2 = wtile(w_b1c2, "w_b1c2")
        tw_b2c1 = wtile(w_b2c1, "w_b2c1")
        tw_b2sk = wtile(w_b2sk, "w_b2sk")
        tw_b2c2 = wtile(w_b2c2, "w_b2c2")
        tw_b3c1 = wtile(w_b3c1, "w_b3c1")
        tw_b3c2 = wtile(w_b3c2, "w_b3c2")
        tident = wtile(ident, "ident")
        tw1 = wtile(w1, "w1")
        tw1x = wtile(w1x, "w1x")
        trhsx = wtile(rhsx, "rhsx")
        tw2t = wtile(w2t, "w2t")
        tb2row = wtile(b2row, "b2row")
        tones1 = wtile(ones1, "ones1")
        tD = wtile(D, "D")
        tcb = wtile(cbias, "cbias")

        # persistent padded activation tiles, double buffered by item parity
        def padded(name, chunks):
            ts_ = []
            for par in range(PAR):
                t = apool.tile([128, chunks * TP], f32r, tag=f"{name}{par}", name=f"{name}{par}")
                for ch in range(chunks):
                    nc.vector.memset(t[:, ch * TP:ch * TP + 1].bitcast(f32), 0.0)
                    nc.vector.memset(t[:, ch * TP + 282:ch * TP + 284].bitcast(f32), 0.0)
                ts_.append(t)
            return ts_

        xp = padded("xp", KC)
        # junk K rows of X chunk 2 (only 15 valid) must be zero
        for par in range(PAR):
            nc.vector.memset(xp[par][:, 2 * TP:3 * TP].bitcast(f32), 0.0)
        mg = [apool.tile([128, KC * 128], f32r, tag=f"mg{par}", name=f"mg{par}") for par in range(PAR)]
        h0p = padded("h0p", 1)
        y1p = padded("y1p", 1)
        h1p = padded("h1p", 1)
        y2p = padded("y2p", 2)
        h2p = padded("h2p", 2)
        y3p = padded("y3p", 2)
        h3p = padded("h3p", 2)

        # pooled feature accumulators (fp32), one column per item
        V = [apool.tile([128, n_items], f32, tag=f"V{c}", name=f"V{c}") for c in range(2)]

        psum_ctr = [0]

        def psum_tile(rows=128, width=N):
            t = pspool.tile([rows, width], f32, tag=f"ps{psum_ctr[0] % 8}",
                            name=f"psum{psum_ctr[0]}")
            psum_ctr[0] += 1
            return t

        def win(tl, ch, k):
            """282-wide window at tap k of padded chunk ch."""
            return tl[:, ch * TP + k: ch * TP + k + N]

        for b in range(n_items):
            par = b % PAR
            xt, mt = xp[par], mg[par]
            # load X chunks into padded positions (cols 1..281 of each chunk)
            for ch in range(KC):
                rows = 128 if ch < 2 else C - 2 * 128
                nc.sync.dma_start(
                    xt[:rows, ch * TP + 1: ch * TP + 1 + T],
                    X[b, ch * 128: ch * 128 + rows, :])
            nc.sync.dma_start(mt[:], Mg[b])

            # ---- fused front matmul: h0 = M_b @ X_b (+ D[:, b]) ----
            p = psum_tile()
            for ch in range(KC):
                nc.tensor.matmul(p[:], mt[:, ch * 128:(ch + 1) * 128],
                                 win(xt, ch, 1), start=(ch == 0), stop=(ch == 2))
            nc.scalar.activation(h0p[par][:, 1:1 + T], p[:, :T], AF.Identity,
                                 bias=tD[:, b:b + 1])

            # ---- block1 conv1: y1 = gelu(conv(h0) + b) ----
            p = psum_tile()
            for k in range(3):
                nc.tensor.matmul(p[:], tw_b1c1[:, k * 128:(k + 1) * 128],
                                 win(h0p[par], 0, k), start=(k == 0), stop=(k == 2))
            nc.scalar.activation(y1p[par][:, 1:1 + T], p[:, :T], AF.Gelu,
                                 bias=tcb[:, 0:1])

            # ---- block1 conv2 + identity residual ----
            p = psum_tile()
            for k in range(3):
                nc.tensor.matmul(p[:], tw_b1c2[:, k * 128:(k + 1) * 128],
                                 win(y1p[par], 0, k), start=(k == 0), stop=False)
            nc.tensor.matmul(p[:], tident[:], win(h0p[par], 0, 1),
                             start=False, stop=True)
            nc.scalar.activation(h1p[par][:, 1:1 + T], p[:, :T], AF.Gelu,
                                 bias=tcb[:, 1:2])

            # ---- block2 conv1 (128 -> 256) ----
            for h in range(2):
                p = psum_tile()
                for k in range(3):
                    nc.tensor.matmul(p[:], tw_b2c1[:, (k * 2 + h) * 128:(k * 2 + h + 1) * 128],
                                     win(h1p[par], 0, k), start=(k == 0), stop=(k == 2))
                nc.scalar.activation(y2p[par][:, h * TP + 1: h * TP + 1 + T],
                                     p[:, :T], AF.Gelu, bias=tcb[:, 2 + h:3 + h])

            # ---- block2 conv2 (256 -> 256) + 1x1 skip from h1 ----
            for h in range(2):
                p = psum_tile()
                first = True
                for k in range(3):
                    for c in range(2):
                        nc.tensor.matmul(
                            p[:], tw_b2c2[:, ((k * 2 + c) * 2 + h) * 128:((k * 2 + c) * 2 + h + 1) * 128],
                            win(y2p[par], c, k), start=first, stop=False)
                        first = False
                nc.tensor.matmul(p[:], tw_b2sk[:, h * 128:(h + 1) * 128],
                                 win(h1p[par], 0, 1), start=False, stop=True)
                nc.scalar.activation(h2p[par][:, h * TP + 1: h * TP + 1 + T],
                                     p[:, :T], AF.Gelu, bias=tcb[:, 4 + h:5 + h])

            # ---- block3 conv1 (256 -> 256) ----
            for h in range(2):
                p = psum_tile()
                first = True
                for k in range(3):
                    for c in range(2):
                        nc.tensor.matmul(
                            p[:], tw_b3c1[:, ((k * 2 + c) * 2 + h) * 128:((k * 2 + c) * 2 + h + 1) * 128],
                            win(h2p[par], c, k), start=first, stop=False)
                        first = False
                nc.scalar.activation(y3p[par][:, h * TP + 1: h * TP + 1 + T],
                                     p[:, :T], AF.Gelu, bias=tcb[:, 6 + h:7 + h])

            # ---- block3 conv2 + identity residual ----
            for h in range(2):
                p = psum_tile()
                first = True
                for k in range(3):
                    for c in range(2):
                        nc.tensor.matmul(
                            p[:], tw_b3c2[:, ((k * 2 + c) * 2 + h) * 128:((k * 2 + c) * 2 + h + 1) * 128],
                            win(y3p[par], c, k), start=first, stop=False)
                        first = False
                nc.tensor.matmul(p[:], tident[:], win(h2p[par], h, 1),
                                 start=False, stop=True)
                nc.scalar.activation(h3p[par][:, h * TP + 1: h * TP + 1 + T],
                                     p[:, :T], AF.Gelu, bias=tcb[:, 8 + h:9 + h])

            # ---- pool over time into V columns ----
            for c in range(2):
                nc.vector.reduce_sum(V[c][:, b:b + 1],
                                     h3p[par][:, c * TP:(c + 1) * TP], axis=AX.X)

        # ---- head (batched over all items of this core) ----
        Vr = [apool.tile([128, n_items], f32r, tag=f"Vr{c}", name=f"Vr{c}") for c in range(2)]
        for c in range(2):
            nc.vector.tensor_copy(Vr[c][:], V[c][:])

        ph = psum_tile(128, n_items)
        for c in range(2):
            nc.tensor.matmul(ph[:], tw1[:, c * 128:(c + 1) * 128], Vr[c][:],
                             start=(c == 0), stop=False)
        nc.tensor.matmul(ph[:], tw1x[:], trhsx[:], start=False, stop=True)
        hmid = apool.tile([128, n_items], f32r, tag="hmid")
        nc.scalar.activation(hmid[:], ph[:], AF.Relu)

        out_sb = apool.tile([n_items, NCLS], f32, tag="out_sb")
        nsplit = [512, 512, 512, NCLS - 3 * 512]
        off = 0
        for w_ in nsplit:
            po = psum_tile(n_items, w_)
            nc.tensor.matmul(po[:], hmid[:], tw2t[:, off:off + w_],
                             start=True, stop=False)
            nc.tensor.matmul(po[:], tones1[:], tb2row[:, off:off + w_],
                             start=False, stop=True)
            nc.vector.tensor_copy(out_sb[:, off:off + w_], po[:])
            off += w_
        nc.sync.dma_start(out[:, :], out_sb[:, :])

        for p_ in (pspool, iopool, apool, wpool):
            p_.release()

    nc.compile()
    return nc


def _preprocess(inputs):
    """Host-side weight folding/packing. Returns (shared weight arrays,
    per-item arrays)."""
    f = np.float64
    attn = inputs["attention"].astype(f)
    attn = attn - attn.max(axis=1, keepdims=True)
    np.exp(attn, out=attn)
    attn /= attn.sum(axis=1, keepdims=True)
    A = inputs["sa_w"].astype(f) @ attn                       # [H, C]
    subj_w = inputs["subj_w"].astype(f)
    M = np.einsum("shk,kc->shc", subj_w, A)                   # [S, H, C]
    # lhsT per subject padded to [KC*128, 128] then chunk-major [128, KC*128]
    MT = np.zeros((NSUBJ, KC * 128, H), np.float32)
    MT[:, :C, :] = np.transpose(M, (0, 2, 1))
    MT = MT.reshape(NSUBJ, KC, 128, H).transpose(0, 2, 1, 3).reshape(NSUBJ, 128, KC * 128)
    # per-item h0 bias: W_s @ sa_b + subj_b[s]
    Dall = (np.einsum("shk,k->sh", subj_w, inputs["sa_b"].astype(f))
            + inputs["subj_b"].astype(f)).astype(np.float32)  # [S, H]

    inv = 1.0 / np.sqrt(1.0 + EPS)

    def fold(w, b, g, be):
        s = g.astype(f) * inv
        wf = w.astype(f) * s[:, None, None]
        bf = s * b.astype(f) + be.astype(f)
        return wf, bf.astype(np.float32)

    def pack_taps(wf, cin_chunks, cout_halves):
        # -> [128, 3*cin_chunks*cout_halves*128], col block (k, c, h)
        O, I, _ = wf.shape
        blocks = []
        for k in range(3):
            for c in range(cin_chunks):
                for h in range(cout_halves):
                    blk = wf[h * 128:(h + 1) * 128, c * 128:(c + 1) * 128, k].T
                    blocks.append(blk.astype(np.float32))
        return np.concatenate(blocks, axis=1)

    w11, b11 = fold(inputs["b1_c1w"], inputs["b1_c1b"], inputs["b1_g1"], inputs["b1_be1"])
    w12, b12 = fold(inputs["b1_c2w"], inputs["b1_c2b"], inputs["b1_g2"], inputs["b1_be2"])
    w21, b21 = fold(inputs["b2_c1w"], inputs["b2_c1b"], inputs["b2_g1"], inputs["b2_be1"])
    w22, b22 = fold(inputs["b2_c2w"], inputs["b2_c2b"], inputs["b2_g2"], inputs["b2_be2"])
    w31, b31 = fold(inputs["b3_c1w"], inputs["b3_c1b"], inputs["b3_g1"], inputs["b3_be1"])
    w32, b32 = fold(inputs["b3_c2w"], inputs["b3_c2b"], inputs["b3_g2"], inputs["b3_be2"])
    skw = inputs["b2_skw"][:, :, 0].astype(np.float32)        # [H2, H]
    skb = inputs["b2_skb"].astype(np.float32)
    b22 = b22 + skb

    cbias = np.zeros((128, 10), np.float32)
    cbias[:, 0] = b11
    cbias[:, 1] = b12
    cbias[:, 2], cbias[:, 3] = b21[:128], b21[128:]
    cbias[:, 4], cbias[:, 5] = b22[:128], b22[128:]
    cbias[:, 6], cbias[:, 7] = b31[:128], b31[128:]
    cbias[:, 8], cbias[:, 9] = b32[:128], b32[128:]

    head_w1 = inputs["head_w1"].astype(f)
    w1pack = np.concatenate(
        [(head_w1[:, c * 128:(c + 1) * 128] / T).T.astype(np.float32) for c in range(2)],
        axis=1)                                               # [128, 256]
    w1x = np.zeros((128, 128), np.float32)
    w1x[:E, :] = head_w1[:, 2 * 128:2 * 128 + E].T
    w1x[E, :] = inputs["head_b1"]
    w2t = inputs["head_w2"].T.astype(np.float32)              # [H, NCLS]
    b2row = inputs["head_b2"].astype(np.float32)[None, :]

    shared = dict(
        cbias=cbias,
        w_b1c1=pack_taps(w11, 1, 1), w_b1c2=pack_taps(w12, 1, 1),
        w_b2c1=pack_taps(w21, 1, 2),
        w_b2sk=np.concatenate([skw[:128].T, skw[128:].T], axis=1).astype(np.float32),
        w_b2c2=pack_taps(w22, 2, 2),
        w_b3c1=pack_taps(w31, 2, 2), w_b3c2=pack_taps(w32, 2, 2),
        ident=np.eye(128, dtype=np.float32),
        w1=w1pack, w1x=w1x, w2t=w2t, b2row=b2row,
    )

    sidx = inputs["subject_idxs"].astype(np.int64)
    Mg = MT[sidx]                                             # [B, 128, KC*128]
    Dcols = Dall[sidx].T.astype(np.float32)                   # [128, B]
    emb = inputs["emb"].astype(np.float32)
    embG = emb[sidx].T                                        # [E, B]
    return shared, Mg, Dcols, embG


def _run(inputs, n_items, n_cores):
    key = (n_items, n_cores)
    if key not in _CACHE:
        _CACHE[key] = _build(n_items, n_cores)
    nc = _CACHE[key]

    shared, Mg, Dcols, embG = _preprocess(inputs)
    X = np.ascontiguousarray(inputs["X"], dtype=np.float32)

    in_maps = []
    for c in range(n_cores):
        lo, hi = c * n_items, (c + 1) * n_items
        rhsx = np.zeros((128, n_items), np.float32)
        rhsx[:E, :] = embG[:, lo:hi]
        rhsx[E, :] = 1.0
        m = dict(shared)
        m["X"] = X[lo:hi]
        m["Mg"] = np.ascontiguousarray(Mg[lo:hi])
        m["D"] = np.ascontiguousarray(Dcols[:, lo:hi])
        m["rhsx"] = rhsx
        m["ones1"] = np.ones((1, n_items), np.float32)
        in_maps.append(m)

    trace = bool(int(os.environ.get("KTRACE", "0")))
    if trace:
        try:
            from antenv.axon_hooks import (get_axon_ntff_profile_hook,
                                           set_axon_ntff_profile_hook)
            if get_axon_ntff_profile_hook() is None:
                from trn_agent_boot.trn_boot import _ntff_profile_via_ctypes
                set_axon_ntff_profile_hook(
                    _ntff_profile_via_ctypes("/opt/axon/libaxon_pjrt.so"))
        except Exception as e:  # dev-only profiling aid
            print(f"(ntff hook unavailable: {e})")
    res = run_bass_kernel_spmd(nc, in_maps, core_ids=list(range(n_cores)),
                               trace=trace)
    outp = np.concatenate([res.results[c]["out"] for c in range(n_cores)], axis=0)
    if trace:
        print(f"HW exec time: {res.exec_time_ns} ns "
              f"(mean {res.mean_exec_time_ns}, max core {res.max_exec_time_core_id})")
    return outp, res


def kernel(**inputs):
    outp, _ = _run(inputs, B // 8, 8)
    return outp


# revision 11
# speedup vs baseline: 1.1082x; 1.1082x over previous
"""Trainium2 Bass kernel for BasicConvClassifierWithSubject.

Strategy: pure data parallel over 8 cores (128 items/core). Per item the whole
network runs as a chain of matmuls on the PE with BN folded into weights,
gelu on ScalarE, residuals accumulated in PSUM via identity/skip matmuls.
Spatial attention + per-subject 1x1 conv fuse on the host into one per-item
[271,128] stationary matrix (gathered by subject), so the device sees a single
K=271 matmul per item for the whole front end. Early stages run in f32r
(tf32-like, full PE rate at N>=256), the heavy 256-channel tail stages in
bf16 (fast weight load).
"""

import os
import numpy as np
import ml_dtypes

import concourse.bass as bass
import concourse.tile as tile
from concourse import bacc, mybir
from concourse.bass_utils import run_bass_kernel_spmd

f32 = mybir.dt.float32
f32r = mybir.dt.float32r
bf16 = mybir.dt.bfloat16
AF = mybir.ActivationFunctionType
AX = mybir.AxisListType

# problem constants (hardcoded per harness contract)
B, C, T = 1024, 271, 281
H, H2, E, NCLS, NSUBJ = 128, 256, 16, 1854, 4
EPS = 1e-5
TP = 284          # padded time length per chunk (zeros at cols 0, 282, 283)
N = 282           # matmul moving size (even, covers cols t..t+281 of pad)
KC = 3            # K chunks for the fused front matmul (271 -> 128+128+15)
PAR = 4           # pipeline depth across items (persistent tile copies)

_CACHE = {}


def _stage_dtypes():
    """Which matmul stages run bf16 vs f32r (KBF16 env knob for experiments)."""
    mode = os.environ.get("KBF16", "late")
    late = {"b2c2", "b3c1", "b3c2"}
    mid = late | {"b2c1", "b2sk"}
    allm = mid | {"front", "b1c1", "b1c2"}
    sel = {"none": set(), "late": late, "mid": mid, "all": allm}[mode]
    return {s: (bf16 if s in sel else f32r)
            for s in ("front", "b1c1", "b1c2", "b2c1", "b2sk", "b2c2", "b3c1", "b3c2")}


def _build(n_items, n_cores, sdt):
    nc = bacc.Bacc("TRN2", target_bir_lowering=False, debug=False,
                   num_devices=n_cores)

    # activation tile dtypes are set by the consuming matmul stage
    xp_dt = sdt["front"]
    h0_dt = sdt["b1c1"]     # also read by b1c2 ident residual
    y1_dt = sdt["b1c2"]
    h1_dt = sdt["b2c1"]     # also read by b2 skip matmul
    y2_dt = sdt["b2c2"]
    h2_dt = sdt["b3c1"]     # also read by b3c2 ident residual
    y3_dt = sdt["b3c2"]

    def din(name, shape, dt=f32r):
        return nc.dram_tensor(name, shape, dt, kind="ExternalInput").ap()

    X = din("X", [n_items, C, T], xp_dt)
    Mg = din("Mg", [n_items, 128, KC * 128], sdt["front"])
    D = din("D", [128, n_items], f32)
    cbias = din("cbias", [128, 10], f32)
    w_b1c1 = din("w_b1c1", [128, 3 * 128], sdt["b1c1"])
    w_b1c2 = din("w_b1c2", [128, 3 * 128], sdt["b1c2"])
    w_b2c1 = din("w_b2c1", [128, 6 * 128], sdt["b2c1"])
    w_b2sk = din("w_b2sk", [128, 2 * 128], sdt["b2sk"])
    w_b2c2 = din("w_b2c2", [128, 12 * 128], sdt["b2c2"])
    w_b3c1 = din("w_b3c1", [128, 12 * 128], sdt["b3c1"])
    w_b3c2 = din("w_b3c2", [128, 12 * 128], sdt["b3c2"])
    ident_r = din("ident_r", [128, 128], sdt["b1c2"])
    ident_h = din("ident_h", [128, 128], sdt["b3c2"])
    w1 = din("w1", [128, 2 * 128])
    w1x = din("w1x", [128, 128])
    rhsx = din("rhsx", [128, n_items])
    w2t = din("w2t", [128, NCLS])
    b2row = din("b2row", [1, NCLS])
    ones1 = din("ones1", [1, n_items])
    out = nc.dram_tensor("out", [n_items, NCLS], f32, kind="ExternalOutput").ap()

    with tile.TileContext(nc) as tc:
        wpool = tc.alloc_tile_pool(name="w", bufs=1)
        apool = tc.alloc_tile_pool(name="a", bufs=1)
        pspool = tc.alloc_tile_pool(name="ps", bufs=1, space="PSUM")

        def wtile(ap_, name):
            t = wpool.tile(list(ap_.shape), ap_.dtype, tag=name, name=name)
            nc.sync.dma_start(t[:], ap_[:])
            return t

        # weights needed from item 0 — load first
        tD = wtile(D, "D")
        tcb = wtile(cbias, "cbias")
        tw_b1c1 = wtile(w_b1c1, "w_b1c1")
        tw_b1c2 = wtile(w_b1c2, "w_b1c2")
        tident_r = wtile(ident_r, "ident_r")
        tw_b2c1 = wtile(w_b2c1, "w_b2c1")
        tw_b2sk = wtile(w_b2sk, "w_b2sk")
        tw_b2c2 = wtile(w_b2c2, "w_b2c2")
        tw_b3c1 = wtile(w_b3c1, "w_b3c1")
        tw_b3c2 = wtile(w_b3c2, "w_b3c2")
        tident_h = wtile(ident_h, "ident_h")

        def zset(ap_):
            nc.vector.memset(ap_.bitcast(f32) if ap_.dtype == f32r else ap_, 0.0)

        def padded(name, chunks, dt):
            ts_ = []
            for par in range(PAR):
                t = apool.tile([128, chunks * TP], dt, tag=f"{name}{par}",
                               name=f"{name}{par}")
                for ch in range(chunks):
                    zset(t[:, ch * TP:ch * TP + 1])
                    zset(t[:, ch * TP + 282:ch * TP + 284])
                ts_.append(t)
            return ts_

        xp = padded("xp", KC, xp_dt)
        for par in range(PAR):
            zset(xp[par][:, 2 * TP:3 * TP])
        mg = [apool.tile([128, KC * 128], sdt["front"], tag=f"mg{par}",
                         name=f"mg{par}") for par in range(PAR)]
        h0p = padded("h0p", 1, h0_dt)
        y1p = padded("y1p", 1, y1_dt)
        h1p = padded("h1p", 1, h1_dt)
        y2p = padded("y2p", 2, y2_dt)
        h2p = padded("h2p", 2, h2_dt)
        y3p = padded("y3p", 2, y3_dt)
        h3p = padded("h3p", 2, f32)

        V = [apool.tile([128, n_items], f32, tag=f"V{c}", name=f"V{c}")
             for c in range(2)]

        psum_ctr = [0]

        def psum_tile(rows=128, width=N):
            t = pspool.tile([rows, width], f32, tag=f"ps{psum_ctr[0] % 8}",
                            name=f"psum{psum_ctr[0]}")
            psum_ctr[0] += 1
            return t

        def win(tl, ch, k):
            return tl[:, ch * TP + k: ch * TP + k + N]

        for b in range(n_items):
            par = b % PAR
            xt, mt = xp[par], mg[par]
            for ch in range(KC):
                rows = 128 if ch < 2 else C - 2 * 128
                nc.sync.dma_start(
                    xt[:rows, ch * TP + 1: ch * TP + 1 + T],
                    X[b, ch * 128: ch * 128 + rows, :])
            nc.sync.dma_start(mt[:], Mg[b])

            # fused front matmul: h0 = M_b @ X_b + D[:, b]
            p = psum_tile()
            for ch in range(KC):
                nc.tensor.matmul(p[:], mt[:, ch * 128:(ch + 1) * 128],
                                 win(xt, ch, 1), start=(ch == 0), stop=(ch == 2))
            nc.vector.tensor_scalar_add(h0p[par][:, 1:1 + T], p[:, :T],
                                        tD[:, b:b + 1])

            # block1 conv1
            p = psum_tile()
            for k in range(3):
                nc.tensor.matmul(p[:], tw_b1c1[:, k * 128:(k + 1) * 128],
                                 win(h0p[par], 0, k), start=(k == 0), stop=(k == 2))
            nc.scalar.activation(y1p[par][:, 1:1 + T], p[:, :T], AF.Gelu,
                                 bias=tcb[:, 0:1])

            # block1 conv2 + identity residual
            p = psum_tile()
            for k in range(3):
                nc.tensor.matmul(p[:], tw_b1c2[:, k * 128:(k + 1) * 128],
                                 win(y1p[par], 0, k), start=(k == 0), stop=False)
            nc.tensor.matmul(p[:], tident_r[:], win(h0p[par], 0, 1),
                             start=False, stop=True)
            nc.scalar.activation(h1p[par][:, 1:1 + T], p[:, :T], AF.Gelu,
                                 bias=tcb[:, 1:2])

            # block2 conv1 (128 -> 256)
            for h in range(2):
                p = psum_tile()
                for k in range(3):
                    nc.tensor.matmul(p[:], tw_b2c1[:, (k * 2 + h) * 128:(k * 2 + h + 1) * 128],
                                     win(h1p[par], 0, k), start=(k == 0), stop=(k == 2))
                nc.scalar.activation(y2p[par][:, h * TP + 1: h * TP + 1 + T],
                                     p[:, :T], AF.Gelu, bias=tcb[:, 2 + h:3 + h])

            # block2 conv2 (256 -> 256) + 1x1 skip from h1
            for h in range(2):
                p = psum_tile()
                first = True
                for k in range(3):
                    for c in range(2):
                        nc.tensor.matmul(
                            p[:], tw_b2c2[:, ((k * 2 + c) * 2 + h) * 128:((k * 2 + c) * 2 + h + 1) * 128],
                            win(y2p[par], c, k), start=first, stop=False)
                        first = False
                nc.tensor.matmul(p[:], tw_b2sk[:, h * 128:(h + 1) * 128],
                                 win(h1p[par], 0, 1), start=False, stop=True)
                nc.scalar.activation(h2p[par][:, h * TP + 1: h * TP + 1 + T],
                                     p[:, :T], AF.Gelu, bias=tcb[:, 4 + h:5 + h])

            # block3 conv1 (256 -> 256)
            for h in range(2):
                p = psum_tile()
                first = True
                for k in range(3):
                    for c in range(2):
                        nc.tensor.matmul(
                            p[:], tw_b3c1[:, ((k * 2 + c) * 2 + h) * 128:((k * 2 + c) * 2 + h + 1) * 128],
                            win(h2p[par], c, k), start=first, stop=False)
                        first = False
                nc.scalar.activation(y3p[par][:, h * TP + 1: h * TP + 1 + T],
                                     p[:, :T], AF.Gelu, bias=tcb[:, 6 + h:7 + h])

            # block3 conv2 + identity residual
            for h in range(2):
                p = psum_tile()
                first = True
                for k in range(3):
                    for c in range(2):
                        nc.tensor.matmul(
                            p[:], tw_b3c2[:, ((k * 2 + c) * 2 + h) * 128:((k * 2 + c) * 2 + h + 1) * 128],
                            win(y3p[par], c, k), start=first, stop=False)
                        first = False
                nc.tensor.matmul(p[:], tident_h[:], win(h2p[par], h, 1),
                                 start=False, stop=True)
                nc.scalar.activation(h3p[par][:, h * TP + 1: h * TP + 1 + T],
                                     p[:, :T], AF.Gelu, bias=tcb[:, 8 + h:9 + h])

            # pool over time into V columns
            for c in range(2):
                nc.vector.reduce_sum(V[c][:, b:b + 1],
                                     h3p[par][:, c * TP:(c + 1) * TP], axis=AX.X)

        # head weights (not needed until the very end)
        tw1 = wtile(w1, "w1")
        tw1x = wtile(w1x, "w1x")
        trhsx = wtile(rhsx, "rhsx")
        tw2t = wtile(w2t, "w2t")
        tb2row = wtile(b2row, "b2row")
        tones1 = wtile(ones1, "ones1")

        Vr = [apool.tile([128, n_items], f32r, tag=f"Vr{c}", name=f"Vr{c}")
              for c in range(2)]
        for c in range(2):
            nc.vector.tensor_copy(Vr[c][:], V[c][:])

        ph = psum_tile(128, n_items)
        for c in range(2):
            nc.tensor.matmul(ph[:], tw1[:, c * 128:(c + 1) * 128], Vr[c][:],
                             start=(c == 0), stop=False)
        nc.tensor.matmul(ph[:], tw1x[:], trhsx[:], start=False, stop=True)
        hmid = apool.tile([128, n_items], f32r, tag="hmid", name="hmid")
        nc.scalar.activation(hmid[:], ph[:], AF.Relu)

        out_sb = apool.tile([n_items, NCLS], f32, tag="out_sb", name="out_sb")
        nsplit = [512, 512, 512, NCLS - 3 * 512]
        off = 0
        for w_ in nsplit:
            po = psum_tile(n_items, w_)
            nc.tensor.matmul(po[:], hmid[:], tw2t[:, off:off + w_],
                             start=True, stop=False)
            nc.tensor.matmul(po[:], tones1[:], tb2row[:, off:off + w_],
                             start=False, stop=True)
            nc.vector.tensor_copy(out_sb[:, off:off + w_], po[:])
            off += w_
        nc.sync.dma_start(out[:, :], out_sb[:, :])

        for p_ in (pspool, apool, wpool):
            p_.release()

    nc.compile()
    return nc


def _preprocess(inputs, sdt):
    f = np.float64

    def npdt(dt):
        return ml_dtypes.bfloat16 if dt == bf16 else np.float32

    attn = inputs["attention"].astype(f)
    attn = attn - attn.max(axis=1, keepdims=True)
    np.exp(attn, out=attn)
    attn /= attn.sum(axis=1, keepdims=True)
    A = inputs["sa_w"].astype(f) @ attn                       # [H, C]
    subj_w = inputs["subj_w"].astype(f)
    M = np.einsum("shk,kc->shc", subj_w, A)                   # [S, H, C]
    MT = np.zeros((NSUBJ, KC * 128, H), np.float32)
    MT[:, :C, :] = np.transpose(M, (0, 2, 1))
    MT = (MT.reshape(NSUBJ, KC, 128, H).transpose(0, 2, 1, 3)
            .reshape(NSUBJ, 128, KC * 128).astype(npdt(sdt["front"])))
    Dall = (np.einsum("shk,k->sh", subj_w, inputs["sa_b"].astype(f))
            + inputs["subj_b"].astype(f)).astype(np.float32)

    inv = 1.0 / np.sqrt(1.0 + EPS)

    def fold(w, b, g, be):
        s = g.astype(f) * inv
        wf = w.astype(f) * s[:, None, None]
        bf_ = s * b.astype(f) + be.astype(f)
        return wf, bf_.astype(np.float32)

    def pack_taps(wf, cin_chunks, cout_halves, dt):
        blocks = []
        for k in range(3):
            for c in range(cin_chunks):
                for h in range(cout_halves):
                    blk = wf[h * 128:(h + 1) * 128, c * 128:(c + 1) * 128, k].T
                    blocks.append(blk)
        return np.concatenate(blocks, axis=1).astype(npdt(dt))

    w11, b11 = fold(inputs["b1_c1w"], inputs["b1_c1b"], inputs["b1_g1"], inputs["b1_be1"])
    w12, b12 = fold(inputs["b1_c2w"], inputs["b1_c2b"], inputs["b1_g2"], inputs["b1_be2"])
    w21, b21 = fold(inputs["b2_c1w"], inputs["b2_c1b"], inputs["b2_g1"], inputs["b2_be1"])
    w22, b22 = fold(inputs["b2_c2w"], inputs["b2_c2b"], inputs["b2_g2"], inputs["b2_be2"])
    w31, b31 = fold(inputs["b3_c1w"], inputs["b3_c1b"], inputs["b3_g1"], inputs["b3_be1"])
    w32, b32 = fold(inputs["b3_c2w"], inputs["b3_c2b"], inputs["b3_g2"], inputs["b3_be2"])
    skw = inputs["b2_skw"][:, :, 0].astype(np.float64)
    skb = inputs["b2_skb"].astype(np.float32)
    b22 = b22 + skb

    cbias = np.zeros((128, 10), np.float32)
    cbias[:, 0] = b11
    cbias[:, 1] = b12
    cbias[:, 2], cbias[:, 3] = b21[:128], b21[128:]
    cbias[:, 4], cbias[:, 5] = b22[:128], b22[128:]
    cbias[:, 6], cbias[:, 7] = b31[:128], b31[128:]
    cbias[:, 8], cbias[:, 9] = b32[:128], b32[128:]

    head_w1 = inputs["head_w1"].astype(f)
    w1pack = np.concatenate(
        [(head_w1[:, c * 128:(c + 1) * 128] / T).T.astype(np.float32) for c in range(2)],
        axis=1)
    w1x = np.zeros((128, 128), np.float32)
    w1x[:E, :] = head_w1[:, 2 * 128:2 * 128 + E].T
    w1x[E, :] = inputs["head_b1"]
    w2t = inputs["head_w2"].T.astype(np.float32)
    b2row = inputs["head_b2"].astype(np.float32)[None, :]

    shared = dict(
        cbias=cbias,
        w_b1c1=pack_taps(w11, 1, 1, sdt["b1c1"]),
        w_b1c2=pack_taps(w12, 1, 1, sdt["b1c2"]),
        w_b2c1=pack_taps(w21, 1, 2, sdt["b2c1"]),
        w_b2sk=np.concatenate([skw[:128].T, skw[128:].T], axis=1).astype(npdt(sdt["b2sk"])),
        w_b2c2=pack_taps(w22, 2, 2, sdt["b2c2"]),
        w_b3c1=pack_taps(w31, 2, 2, sdt["b3c1"]),
        w_b3c2=pack_taps(w32, 2, 2, sdt["b3c2"]),
        ident_r=np.eye(128, dtype=npdt(sdt["b1c2"])),
        ident_h=np.eye(128, dtype=npdt(sdt["b3c2"])),
        w1=w1pack, w1x=w1x, w2t=w2t, b2row=b2row,
    )

    sidx = inputs["subject_idxs"].astype(np.int64)
    Mg = MT[sidx]
    Dcols = Dall[sidx].T.astype(np.float32)
    emb = inputs["emb"].astype(np.float32)
    embG = emb[sidx].T
    return shared, Mg, Dcols, embG


def _run(inputs, n_items, n_cores):
    sdt = _stage_dtypes()
    key = (n_items, n_cores, tuple(sorted((k, str(v)) for k, v in sdt.items())))
    if key not in _CACHE:
        _CACHE[key] = _build(n_items, n_cores, sdt)
    nc = _CACHE[key]

    shared, Mg, Dcols, embG = _preprocess(inputs, sdt)
    xdt = ml_dtypes.bfloat16 if sdt["front"] == bf16 else np.float32
    X = np.ascontiguousarray(inputs["X"], dtype=xdt)

    in_maps = []
    for c in range(n_cores):
        lo, hi = c * n_items, (c + 1) * n_items
        rhsx = np.zeros((128, n_items), np.float32)
        rhsx[:E, :] = embG[:, lo:hi]
        rhsx[E, :] = 1.0
        m = dict(shared)
        m["X"] = X[lo:hi]
        m["Mg"] = np.ascontiguousarray(Mg[lo:hi])
        m["D"] = np.ascontiguousarray(Dcols[:, lo:hi])
        m["rhsx"] = rhsx
        m["ones1"] = np.ones((1, n_items), np.float32)
        in_maps.append(m)

    trace = bool(int(os.environ.get("KTRACE", "0")))
    if trace:
        try:
            from antenv.axon_hooks import (get_axon_ntff_profile_hook,
                                           set_axon_ntff_profile_hook)
            if get_axon_ntff_profile_hook() is None:
                from trn_agent_boot.trn_boot import _ntff_profile_via_ctypes
                set_axon_ntff_profile_hook(
                    _ntff_profile_via_ctypes("/opt/axon/libaxon_pjrt.so"))
        except Exception as e:  # dev-only profiling aid
            print(f"(ntff hook unavailable: {e})")
    res = run_bass_kernel_spmd(nc, in_maps, core_ids=list(range(n_cores)),
                               trace=trace)
    outp = np.concatenate([res.results[c]["out"] for c in range(n_cores)], axis=0)
    if trace:
        print(f"HW exec time: {res.exec_time_ns} ns "
              f"(mean {res.mean_exec_time_ns}, max core {res.max_exec_time_core_id})")
    return outp, res


def kernel(**inputs):
    outp, _ = _run(inputs, B // 8, 8)
    return outp


# revision 13
# speedup vs baseline: 1.3288x; 1.1991x over previous
"""Trainium2 Bass kernel for BasicConvClassifierWithSubject.

Strategy: pure data parallel over 8 cores (128 items/core). Per item the whole
network runs as a chain of matmuls on the PE with BN folded into weights,
gelu on ScalarE, residuals accumulated in PSUM via identity/skip matmuls.
Spatial attention + per-subject 1x1 conv fuse on the host into one per-item
[271,128] stationary matrix (gathered by subject), so the device sees a single
K=271 matmul per item for the whole front end. Early stages run in f32r
(tf32-like, full PE rate at N>=256), the heavy 256-channel tail stages in
bf16 (fast weight load).
"""

import os
import numpy as np
import ml_dtypes

import concourse.bass as bass
import concourse.tile as tile
from concourse import bacc, mybir
from concourse.bass_utils import run_bass_kernel_spmd

f32 = mybir.dt.float32
f32r = mybir.dt.float32r
bf16 = mybir.dt.bfloat16
AF = mybir.ActivationFunctionType
AX = mybir.AxisListType

# problem constants (hardcoded per harness contract)
B, C, T = 1024, 271, 281
H, H2, E, NCLS, NSUBJ = 128, 256, 16, 1854, 4
EPS = 1e-5
TP = 284          # padded time length per chunk (zeros at cols 0, 282, 283)
N = 282           # matmul moving size (even, covers cols t..t+281 of pad)
KC = 3            # K chunks for the fused front matmul (271 -> 128+128+15)
PAR = 6           # pipeline depth across items (persistent tile copies)

_CACHE = {}


def _stage_dtypes():
    """Which matmul stages run bf16 vs f32r (KBF16 env knob for experiments)."""
    mode = os.environ.get("KBF16", "mid")
    late = {"b2c2", "b3c1", "b3c2"}
    mid = late | {"b2c1", "b2sk"}
    allm = mid | {"front", "b1c1", "b1c2"}
    sel = {"none": set(), "late": late, "mid": mid, "all": allm}[mode]
    return {s: (bf16 if s in sel else f32r)
            for s in ("front", "b1c1", "b1c2", "b2c1", "b2sk", "b2c2", "b3c1", "b3c2")}


def _build(n_items, n_cores, sdt):
    nc = bacc.Bacc("TRN2", target_bir_lowering=False, debug=False,
                   num_devices=n_cores)

    # activation tile dtypes are set by the consuming matmul stage
    xp_dt = sdt["front"]
    h0_dt = sdt["b1c1"]     # also read by b1c2 ident residual
    y1_dt = sdt["b1c2"]
    h1_dt = sdt["b2c1"]     # also read by b2 skip matmul
    y2_dt = sdt["b2c2"]
    h2_dt = sdt["b3c1"]     # also read by b3c2 ident residual
    y3_dt = sdt["b3c2"]

    def din(name, shape, dt=f32r):
        return nc.dram_tensor(name, shape, dt, kind="ExternalInput").ap()

    X = din("X", [n_items, C, T], xp_dt)
    Mg = din("Mg", [n_items, 128, KC * 128], sdt["front"])
    D = din("D", [128, n_items], f32)
    cbias = din("cbias", [128, 10], f32)
    w_b1c1 = din("w_b1c1", [128, 3 * 128], sdt["b1c1"])
    w_b1c2 = din("w_b1c2", [128, 3 * 128], sdt["b1c2"])
    w_b2c1 = din("w_b2c1", [128, 6 * 128], sdt["b2c1"])
    w_b2sk = din("w_b2sk", [128, 2 * 128], sdt["b2sk"])
    w_b2c2 = din("w_b2c2", [128, 12 * 128], sdt["b2c2"])
    w_b3c1 = din("w_b3c1", [128, 12 * 128], sdt["b3c1"])
    w_b3c2 = din("w_b3c2", [128, 12 * 128], sdt["b3c2"])
    ident_r = din("ident_r", [128, 128], sdt["b1c2"])
    ident_h = din("ident_h", [128, 128], sdt["b3c2"])
    w1 = din("w1", [128, 2 * 128])
    w1x = din("w1x", [128, 128])
    rhsx = din("rhsx", [128, n_items])
    w2t = din("w2t", [128, NCLS])
    b2row = din("b2row", [1, NCLS])
    ones1 = din("ones1", [1, n_items])
    out = nc.dram_tensor("out", [n_items, NCLS], f32, kind="ExternalOutput").ap()

    with tile.TileContext(nc) as tc:
        wpool = tc.alloc_tile_pool(name="w", bufs=1)
        apool = tc.alloc_tile_pool(name="a", bufs=1)
        pspool = tc.alloc_tile_pool(name="ps", bufs=1, space="PSUM")

        def wtile(ap_, name):
            t = wpool.tile(list(ap_.shape), ap_.dtype, tag=name, name=name)
            nc.sync.dma_start(t[:], ap_[:])
            return t

        # weights needed from item 0 — load first
        tD = wtile(D, "D")
        tcb = wtile(cbias, "cbias")
        tw_b1c1 = wtile(w_b1c1, "w_b1c1")
        tw_b1c2 = wtile(w_b1c2, "w_b1c2")
        tident_r = wtile(ident_r, "ident_r")
        tw_b2c1 = wtile(w_b2c1, "w_b2c1")
        tw_b2sk = wtile(w_b2sk, "w_b2sk")
        tw_b2c2 = wtile(w_b2c2, "w_b2c2")
        tw_b3c1 = wtile(w_b3c1, "w_b3c1")
        tw_b3c2 = wtile(w_b3c2, "w_b3c2")
        tident_h = wtile(ident_h, "ident_h")

        def zset(ap_):
            nc.vector.memset(ap_.bitcast(f32) if ap_.dtype == f32r else ap_, 0.0)

        def padded(name, chunks, dt):
            ts_ = []
            for par in range(PAR):
                t = apool.tile([128, chunks * TP], dt, tag=f"{name}{par}",
                               name=f"{name}{par}")
                for ch in range(chunks):
                    zset(t[:, ch * TP:ch * TP + 1])
                    zset(t[:, ch * TP + 282:ch * TP + 284])
                ts_.append(t)
            return ts_

        xp = padded("xp", KC, xp_dt)
        for par in range(PAR):
            zset(xp[par][:, 2 * TP:3 * TP])
        mg = [apool.tile([128, KC * 128], sdt["front"], tag=f"mg{par}",
                         name=f"mg{par}") for par in range(PAR)]
        h0p = padded("h0p", 1, h0_dt)
        y1p = padded("y1p", 1, y1_dt)
        h1p = padded("h1p", 1, h1_dt)
        y2p = padded("y2p", 2, y2_dt)
        h2p = padded("h2p", 2, h2_dt)
        y3p = padded("y3p", 2, y3_dt)
        h3p = padded("h3p", 2, f32)

        V = [apool.tile([128, n_items], f32, tag=f"V{c}", name=f"V{c}")
             for c in range(2)]

        psum_ctr = [0]

        def psum_tile(rows=128, width=N):
            t = pspool.tile([rows, width], f32, tag=f"ps{psum_ctr[0] % 8}",
                            name=f"psum{psum_ctr[0]}")
            psum_ctr[0] += 1
            return t

        def win(tl, ch, k):
            return tl[:, ch * TP + k: ch * TP + k + N]

        def st_load(b, par):
            xt, mt = xp[par], mg[par]
            for ch in range(KC):
                rows = 128 if ch < 2 else C - 2 * 128
                nc.sync.dma_start(
                    xt[:rows, ch * TP + 1: ch * TP + 1 + T],
                    X[b, ch * 128: ch * 128 + rows, :])
            nc.sync.dma_start(mt[:], Mg[b])

        def st_front(b, par):
            # fused front matmul: h0 = M_b @ X_b + D[:, b]
            p = psum_tile()
            for ch in range(KC):
                nc.tensor.matmul(p[:], mg[par][:, ch * 128:(ch + 1) * 128],
                                 win(xp[par], ch, 1), start=(ch == 0), stop=(ch == 2))
            nc.vector.tensor_scalar_add(h0p[par][:, 1:1 + T], p[:, :T],
                                        tD[:, b:b + 1])

        def st_b1c1(b, par):
            p = psum_tile()
            for k in range(3):
                nc.tensor.matmul(p[:], tw_b1c1[:, k * 128:(k + 1) * 128],
                                 win(h0p[par], 0, k), start=(k == 0), stop=(k == 2))
            nc.scalar.activation(y1p[par][:, 1:1 + T], p[:, :T], AF.Gelu,
                                 bias=tcb[:, 0:1])

        def st_b1c2(b, par):
            # conv2 + identity residual
            p = psum_tile()
            for k in range(3):
                nc.tensor.matmul(p[:], tw_b1c2[:, k * 128:(k + 1) * 128],
                                 win(y1p[par], 0, k), start=(k == 0), stop=False)
            nc.tensor.matmul(p[:], tident_r[:], win(h0p[par], 0, 1),
                             start=False, stop=True)
            nc.scalar.activation(h1p[par][:, 1:1 + T], p[:, :T], AF.Gelu,
                                 bias=tcb[:, 1:2])

        def st_b2c1(b, par, h):
            p = psum_tile()
            for k in range(3):
                nc.tensor.matmul(p[:], tw_b2c1[:, (k * 2 + h) * 128:(k * 2 + h + 1) * 128],
                                 win(h1p[par], 0, k), start=(k == 0), stop=(k == 2))
            nc.scalar.activation(y2p[par][:, h * TP + 1: h * TP + 1 + T],
                                 p[:, :T], AF.Gelu, bias=tcb[:, 2 + h:3 + h])

        def st_b2c2(b, par, h):
            p = psum_tile()
            first = True
            for k in range(3):
                for c in range(2):
                    nc.tensor.matmul(
                        p[:], tw_b2c2[:, ((k * 2 + c) * 2 + h) * 128:((k * 2 + c) * 2 + h + 1) * 128],
                        win(y2p[par], c, k), start=first, stop=False)
                    first = False
            nc.tensor.matmul(p[:], tw_b2sk[:, h * 128:(h + 1) * 128],
                             win(h1p[par], 0, 1), start=False, stop=True)
            nc.scalar.activation(h2p[par][:, h * TP + 1: h * TP + 1 + T],
                                 p[:, :T], AF.Gelu, bias=tcb[:, 4 + h:5 + h])

        def st_b3c1(b, par, h):
            p = psum_tile()
            first = True
            for k in range(3):
                for c in range(2):
                    nc.tensor.matmul(
                        p[:], tw_b3c1[:, ((k * 2 + c) * 2 + h) * 128:((k * 2 + c) * 2 + h + 1) * 128],
                        win(h2p[par], c, k), start=first, stop=False)
                    first = False
            nc.scalar.activation(y3p[par][:, h * TP + 1: h * TP + 1 + T],
                                 p[:, :T], AF.Gelu, bias=tcb[:, 6 + h:7 + h])

        def st_b3c2(b, par, h):
            p = psum_tile()
            first = True
            for k in range(3):
                for c in range(2):
                    nc.tensor.matmul(
                        p[:], tw_b3c2[:, ((k * 2 + c) * 2 + h) * 128:((k * 2 + c) * 2 + h + 1) * 128],
                        win(y3p[par], c, k), start=first, stop=False)
                    first = False
            nc.tensor.matmul(p[:], tident_h[:], win(h2p[par], h, 1),
                             start=False, stop=True)
            nc.scalar.activation(h3p[par][:, h * TP + 1: h * TP + 1 + T],
                                 p[:, :T], AF.Gelu, bias=tcb[:, 8 + h:9 + h])

        def st_pool(b, par):
            for c in range(2):
                nc.vector.reduce_sum(V[c][:, b:b + 1],
                                     h3p[par][:, c * TP:(c + 1) * TP], axis=AX.X)

        # Stage-interleaved emission: within a group of G items, emit each
        # stage for all items before moving to the next stage, so the PE has
        # another item's matmuls to run while ScalarE drains a stage.
        stages = ([st_front, st_b1c1, st_b1c2]
                  + [lambda b, par, h=h, f=f: f(b, par, h)
                     for f in (st_b2c1, st_b2c2, st_b3c1, st_b3c2) for h in range(2)]
                  + [st_pool])
        G = int(os.environ.get("KGROUP", "3"))
        for g0 in range(0, n_items, G):
            grp = [(b, b % PAR) for b in range(g0, min(g0 + G, n_items))]
            for b, par in grp:
                st_load(b, par)
            for sf in stages:
                for b, par in grp:
                    sf(b, par)

        # head weights (not needed until the very end)
        tw1 = wtile(w1, "w1")
        tw1x = wtile(w1x, "w1x")
        trhsx = wtile(rhsx, "rhsx")
        tw2t = wtile(w2t, "w2t")
        tb2row = wtile(b2row, "b2row")
        tones1 = wtile(ones1, "ones1")

        Vr = [apool.tile([128, n_items], f32r, tag=f"Vr{c}", name=f"Vr{c}")
              for c in range(2)]
        for c in range(2):
            nc.vector.tensor_copy(Vr[c][:], V[c][:])

        ph = psum_tile(128, n_items)
        for c in range(2):
            nc.tensor.matmul(ph[:], tw1[:, c * 128:(c + 1) * 128], Vr[c][:],
                             start=(c == 0), stop=False)
        nc.tensor.matmul(ph[:], tw1x[:], trhsx[:], start=False, stop=True)
        hmid = apool.tile([128, n_items], f32r, tag="hmid", name="hmid")
        nc.scalar.activation(hmid[:], ph[:], AF.Relu)

        out_sb = apool.tile([n_items, NCLS], f32, tag="out_sb", name="out_sb")
        nsplit = [512, 512, 512, NCLS - 3 * 512]
        off = 0
        for w_ in nsplit:
            po = psum_tile(n_items, w_)
            nc.tensor.matmul(po[:], hmid[:], tw2t[:, off:off + w_],
                             start=True, stop=False)
            nc.tensor.matmul(po[:], tones1[:], tb2row[:, off:off + w_],
                             start=False, stop=True)
            nc.vector.tensor_copy(out_sb[:, off:off + w_], po[:])
            off += w_
        nc.sync.dma_start(out[:, :], out_sb[:, :])

        for p_ in (pspool, apool, wpool):
            p_.release()

    nc.compile()
    return nc


def _preprocess(inputs, sdt):
    f = np.float64

    def npdt(dt):
        return ml_dtypes.bfloat16 if dt == bf16 else np.float32

    attn = inputs["attention"].astype(f)
    attn = attn - attn.max(axis=1, keepdims=True)
    np.exp(attn, out=attn)
    attn /= attn.sum(axis=1, keepdims=True)
    A = inputs["sa_w"].astype(f) @ attn                       # [H, C]
    subj_w = inputs["subj_w"].astype(f)
    M = np.einsum("shk,kc->shc", subj_w, A)                   # [S, H, C]
    MT = np.zeros((NSUBJ, KC * 128, H), np.float32)
    MT[:, :C, :] = np.transpose(M, (0, 2, 1))
    MT = (MT.reshape(NSUBJ, KC, 128, H).transpose(0, 2, 1, 3)
            .reshape(NSUBJ, 128, KC * 128).astype(npdt(sdt["front"])))
    Dall = (np.einsum("shk,k->sh", subj_w, inputs["sa_b"].astype(f))
            + inputs["subj_b"].astype(f)).astype(np.float32)

    inv = 1.0 / np.sqrt(1.0 + EPS)

    def fold(w, b, g, be):
        s = g.astype(f) * inv
        wf = w.astype(f) * s[:, None, None]
        bf_ = s * b.astype(f) + be.astype(f)
        return wf, bf_.astype(np.float32)

    def pack_taps(wf, cin_chunks, cout_halves, dt):
        blocks = []
        for k in range(3):
            for c in range(cin_chunks):
                for h in range(cout_halves):
                    blk = wf[h * 128:(h + 1) * 128, c * 128:(c + 1) * 128, k].T
                    blocks.append(blk)
        return np.concatenate(blocks, axis=1).astype(npdt(dt))

    w11, b11 = fold(inputs["b1_c1w"], inputs["b1_c1b"], inputs["b1_g1"], inputs["b1_be1"])
    w12, b12 = fold(inputs["b1_c2w"], inputs["b1_c2b"], inputs["b1_g2"], inputs["b1_be2"])
    w21, b21 = fold(inputs["b2_c1w"], inputs["b2_c1b"], inputs["b2_g1"], inputs["b2_be1"])
    w22, b22 = fold(inputs["b2_c2w"], inputs["b2_c2b"], inputs["b2_g2"], inputs["b2_be2"])
    w31, b31 = fold(inputs["b3_c1w"], inputs["b3_c1b"], inputs["b3_g1"], inputs["b3_be1"])
    w32, b32 = fold(inputs["b3_c2w"], inputs["b3_c2b"], inputs["b3_g2"], inputs["b3_be2"])
    skw = inputs["b2_skw"][:, :, 0].astype(np.float64)
    skb = inputs["b2_skb"].astype(np.float32)
    b22 = b22 + skb

    cbias = np.zeros((128, 10), np.float32)
    cbias[:, 0] = b11
    cbias[:, 1] = b12
    cbias[:, 2], cbias[:, 3] = b21[:128], b21[128:]
    cbias[:, 4], cbias[:, 5] = b22[:128], b22[128:]
    cbias[:, 6], cbias[:, 7] = b31[:128], b31[128:]
    cbias[:, 8], cbias[:, 9] = b32[:128], b32[128:]

    head_w1 = inputs["head_w1"].astype(f)
    w1pack = np.concatenate(
        [(head_w1[:, c * 128:(c + 1) * 128] / T).T.astype(np.float32) for c in range(2)],
        axis=1)
    w1x = np.zeros((128, 128), np.float32)
    w1x[:E, :] = head_w1[:, 2 * 128:2 * 128 + E].T
    w1x[E, :] = inputs["head_b1"]
    w2t = inputs["head_w2"].T.astype(np.float32)
    b2row = inputs["head_b2"].astype(np.float32)[None, :]

    shared = dict(
        cbias=cbias,
        w_b1c1=pack_taps(w11, 1, 1, sdt["b1c1"]),
        w_b1c2=pack_taps(w12, 1, 1, sdt["b1c2"]),
        w_b2c1=pack_taps(w21, 1, 2, sdt["b2c1"]),
        w_b2sk=np.concatenate([skw[:128].T, skw[128:].T], axis=1).astype(npdt(sdt["b2sk"])),
        w_b2c2=pack_taps(w22, 2, 2, sdt["b2c2"]),
        w_b3c1=pack_taps(w31, 2, 2, sdt["b3c1"]),
        w_b3c2=pack_taps(w32, 2, 2, sdt["b3c2"]),
        ident_r=np.eye(128, dtype=npdt(sdt["b1c2"])),
        ident_h=np.eye(128, dtype=npdt(sdt["b3c2"])),
        w1=w1pack, w1x=w1x, w2t=w2t, b2row=b2row,
    )

    sidx = inputs["subject_idxs"].astype(np.int64)
    Mg = MT[sidx]
    Dcols = Dall[sidx].T.astype(np.float32)
    emb = inputs["emb"].astype(np.float32)
    embG = emb[sidx].T
    return shared, Mg, Dcols, embG


def _run(inputs, n_items, n_cores):
    sdt = _stage_dtypes()
    key = (n_items, n_cores, tuple(sorted((k, str(v)) for k, v in sdt.items())))
    if key not in _CACHE:
        _CACHE[key] = _build(n_items, n_cores, sdt)
    nc = _CACHE[key]

    shared, Mg, Dcols, embG = _preprocess(inputs, sdt)
    xdt = ml_dtypes.bfloat16 if sdt["front"] == bf16 else np.float32
    X = np.ascontiguousarray(inputs["X"], dtype=xdt)

    in_maps = []
    for c in range(n_cores):
        lo, hi = c * n_items, (c + 1) * n_items
        rhsx = np.zeros((128, n_items), np.float32)
        rhsx[:E, :] = embG[:, lo:hi]
        rhsx[E, :] = 1.0
        m = dict(shared)
        m["X"] = X[lo:hi]
        m["Mg"] = np.ascontiguousarray(Mg[lo:hi])
        m["D"] = np.ascontiguousarray(Dcols[:, lo:hi])
        m["rhsx"] = rhsx
        m["ones1"] = np.ones((1, n_items), np.float32)
        in_maps.append(m)

    trace = bool(int(os.environ.get("KTRACE", "0")))
    if trace:
        try:
            from antenv.axon_hooks import (get_axon_ntff_profile_hook,
                                           set_axon_ntff_profile_hook)
            if get_axon_ntff_profile_hook() is None:
                from trn_agent_boot.trn_boot import _ntff_profile_via_ctypes
                set_axon_ntff_profile_hook(
                    _ntff_profile_via_ctypes("/opt/axon/libaxon_pjrt.so"))
        except Exception as e:  # dev-only profiling aid
            print(f"(ntff hook unavailable: {e})")
    res = run_bass_kernel_spmd(nc, in_maps, core_ids=list(range(n_cores)),
                               trace=trace)
    outp = np.concatenate([res.results[c]["out"] for c in range(n_cores)], axis=0)
    if trace:
        print(f"HW exec time: {res.exec_time_ns} ns "
              f"(mean {res.mean_exec_time_ns}, max core {res.max_exec_time_core_id})")
    return outp, res


def kernel(**inputs):
    outp, _ = _run(inputs, B // 8, 8)
    return outp
